# revision 15
# baseline (speedup 1.0000x reference)
"""Trainium2 Bass kernel for nn_Aggregation (involution-style local aggregation).

out[b, g*64+cw, ho, wo] = sum_{i,j in 5x5} xpad[b, g*64+cw, ho+i, wo+j]
                          * weight[b, cw, i*5+j, ho*64+wo]

Data-parallel over batch: 16 samples -> 8 NeuronCores, 2 samples/core.
Per core:
  - DVE computes the 25 shifted elementwise products (batched 5 window
    shifts per tensor_tensor via an overlapping access pattern),
  - TensorE accumulates them into PSUM with identity-stationary f32r
    matmuls (1 cycle/row),
  - ScalarE evacuates PSUM -> SBUF, DMA writes back.
"""

import os
import sys

import numpy as np

sys.path.insert(0, "/opt/trn_rl_repo")

# Problem constants (hardcoded per contract)
B, C, H, W = 16, 512, 64, 64
CW, KK, KS = 64, 25, 5
PAD = 2
NCORES = 8
S = B // NCORES          # samples per core = 2
PADH = H + 2 * PAD       # 68
NBLK = C // 128          # 4 channel blocks of 128 (each = 2 share-groups)
ROWS = 8                 # output rows per chunk
CHUNK = ROWS * W         # 512 positions = 1 PSUM bank of fp32
NCHUNK = H // ROWS       # 8 chunks per sample

_STATE = {}


def _build_nc():
    import concourse.bass as bass
    import concourse.bacc as bacc
    import concourse.tile as tile
    from concourse import mybir

    f32 = mybir.dt.float32
    f32r = mybir.dt.float32r

    nc = bacc.Bacc("TRN2", target_bir_lowering=False, debug=False, num_devices=NCORES)
    x_in = nc.declare_dram_parameter("input", [S, C, H, W], f32, isOutput=False)
    w_in = nc.declare_dram_parameter("weight", [S, CW, KK, H, W], f32, isOutput=False)
    id_in = nc.declare_dram_parameter("ident", [128, 128], f32, isOutput=False)
    out = nc.declare_dram_parameter("out", [S, C, H, W], f32, isOutput=True)

    with tile.TileContext(nc) as tc:
        with (
            tc.tile_pool(name="const", bufs=1) as constp,
            tc.tile_pool(name="xp", bufs=1) as xpp,
            tc.tile_pool(name="wt", bufs=2) as wtp,
            tc.tile_pool(name="tmp", bufs=2) as tmpp,
            tc.tile_pool(name="ost", bufs=2) as ostp,
            tc.tile_pool(name="ps", bufs=4, space="PSUM") as psp,
        ):
            ident = constp.tile([128, 128], f32)
            nc.sync.dma_start(ident[:], id_in[:])

            # Persistent padded-input tiles, one per channel block. Borders
            # are zeroed once; only the interior is rewritten per sample.
            xp = []
            for cb in range(NBLK):
                t = xpp.tile([128, PADH, PADH], f32, tag=f"xp{cb}")
                nc.vector.memset(t[:], 0.0)
                xp.append(t)

            for b in range(S):
                for cb in range(NBLK):
                    nc.sync.dma_start(
                        xp[cb][:, PAD:PAD + H, PAD:PAD + W],
                        x_in[b, cb * 128:(cb + 1) * 128],
                    )
                for k in range(NCHUNK):
                    wt = wtp.tile([128, KK, ROWS, W], f32)
                    # weight rows for this chunk on partitions 0..63, then
                    # duplicated to 64..127 (channel blocks span 2 groups
                    # sharing the same cw range).
                    nc.sync.dma_start(
                        wt[0:64], w_in[b, :, :, k * ROWS:(k + 1) * ROWS, :]
                    )
                    nc.sync.dma_start(wt[64:128], wt[0:64])
                    for cb in range(NBLK):
                        ps = psp.tile([128, ROWS, W], f32)
                        for j in range(KS):
                            t = tmpp.tile([128, KS, ROWS, W], f32)
                            # x window, batched over the 5 vertical shifts i:
                            # dims (i:5 @ PADH, r:ROWS @ PADH, c:W @ 1),
                            # base offset = (k*ROWS)*PADH + j
                            sl = xp[cb][:, k * ROWS:k * ROWS + ROWS, j:j + W]
                            xov = bass.AP(
                                sl.tensor, sl.offset,
                                [list(sl.ap[0]), [PADH, KS], [PADH, ROWS], [1, W]],
                            )
                            # weight idx = i*5+j for i in 0..5:
                            # offset j*ROWS*W, stride 5*ROWS*W over i
                            wsl = wt[:, j]
                            wov = bass.AP(
                                wsl.tensor, wsl.offset,
                                [list(wsl.ap[0]), [KS * ROWS * W, KS], [W, ROWS], [1, W]],
                            )
                            nc.vector.tensor_mul(t[:], xov, wov)
                            for i in range(KS):
                                idx = i * KS + j
                                nc.tensor.matmul(
                                    ps[:],
                                    ident[:].bitcast(f32r),
                                    t[:, i].bitcast(f32r),
                                    start=(j == 0 and i == 0),
                                    stop=(j == KS - 1 and i == KS - 1),
                                )
                        o = ostp.tile([128, ROWS, W], f32)
                        nc.scalar.copy(o[:], ps[:])
                        nc.sync.dma_start(
                            out[b, cb * 128:(cb + 1) * 128, k * ROWS:(k + 1) * ROWS, :],
                            o[:],
                        )
    nc.compile()
    return nc


def _build_nc_bf16():
    """bf16-products variant.

    - DVE tensor_tensor runs in 2x_1P mode (2 elem/cycle/lane): every operand
      is bf16, innermost stride 1, 4B-aligned. Odd horizontal shifts j break
      4B alignment, so a second copy of the input, stored shifted by one
      element, serves the odd-j windows.
    - The host supplies the input pre-padded (zero borders, 68x68 per image)
      and pre-cast to bf16 as [S, C, 68*68+1]; the kernel streams it in
      per-chunk row-halo tiles, fully contiguous for both shifted copies.
    - Products are bf16; accumulation over the 25 taps stays exact in fp32
      PSUM via identity-stationary matmuls (1 cycle/row, <=512 cols/bank).
    - First/last chunks are half-size so the pipeline fills and drains fast.
    """
    import concourse.bass as bass
    import concourse.bacc as bacc
    import concourse.tile as tile
    from concourse import mybir

    f32 = mybir.dt.float32
    bf16 = mybir.dt.bfloat16
    NPAD = PADH * PADH   # 4624
    CHUNKS = [8, 16, 16, 16, 8]  # output rows per chunk (sums to H)

    nc = bacc.Bacc("TRN2", target_bir_lowering=False, debug=False, num_devices=NCORES)
    x_in = nc.declare_dram_parameter("input", [S, C, NPAD + 1], bf16, isOutput=False)
    w_in = nc.declare_dram_parameter("weight", [S, CW, KK, H, W], bf16, isOutput=False)
    id_in = nc.declare_dram_parameter("ident", [128, 128], bf16, isOutput=False)
    out = nc.declare_dram_parameter("out", [S, C, H, W], f32, isOutput=True)

    with tile.TileContext(nc) as tc:
        with (
            tc.tile_pool(name="const", bufs=1) as constp,
            tc.tile_pool(name="xp", bufs=2) as xpp,
            tc.tile_pool(name="wt", bufs=2) as wtp,
            tc.tile_pool(name="tmp", bufs=3) as tmpp,
            tc.tile_pool(name="ost", bufs=2) as ostp,
            tc.tile_pool(name="ps", bufs=3, space="PSUM") as psp,
        ):
            ident = constp.tile([128, 128], bf16)
            nc.sync.dma_start(ident[:], id_in[:])

            for b in range(S):
                r0 = 0
                for k, R in enumerate(CHUNKS):
                    row0 = r0 * PADH
                    halo = (R + KS - 1) * PADH
                    xpa, xpb = [], []
                    for cb in range(NBLK):
                        csl = slice(cb * 128, (cb + 1) * 128)
                        ta = xpp.tile([128, halo], bf16, tag=f"xpa{cb}", name=f"xpa{cb}")
                        nc.sync.dma_start(ta[:], x_in[b, csl, row0:row0 + halo])
                        tb = xpp.tile([128, halo], bf16, tag=f"xpb{cb}", name=f"xpb{cb}")
                        nc.scalar.dma_start(tb[:], x_in[b, csl, row0 + 1:row0 + 1 + halo])
                        xpa.append(ta)
                        xpb.append(tb)
                    wt = wtp.tile([128, KK, R, W], bf16, name="wt", tag="wt")
                    nc.sync.dma_start(wt[0:64], w_in[b, :, :, r0:r0 + R, :])
                    nc.gpsimd.dma_start(wt[64:128], wt[0:64])
                    # split the 25-tap accumulation into <=512-column PSUM banks
                    halves = [(0, R)] if R * W <= 512 else [(0, R // 2), (R // 2, R)]
                    for cb in range(NBLK):
                        ps = psp.tile([128, R, W], f32, name="ps", tag="ps")
                        for j in range(KS):
                            t = tmpp.tile([128, KS, R, W], bf16, name="t", tag="t")
                            if j % 2 == 0:
                                xt = xpa[cb][:]
                                base = xt.offset + j
                            else:
                                xt = xpb[cb][:]
                                base = xt.offset + (j - 1)
                            xov = bass.AP(
                                xt.tensor, base,
                                [list(xt.ap[0]), [PADH, KS], [PADH, R], [1, W]],
                            )
                            wsl = wt[:, j]
                            wov = bass.AP(
                                wsl.tensor, wsl.offset,
                                [list(wsl.ap[0]), [KS * R * W, KS], [W, R], [1, W]],
                            )
                            nc.vector.tensor_mul(t[:], xov, wov)
                            for h0, h1 in halves:
                                for i in range(KS):
                                    nc.tensor.matmul(
                                        ps[:, h0:h1],
                                        ident[:],
                                        t[:, i, h0:h1],
                                        start=(j == 0 and i == 0),
                                        stop=(j == KS - 1 and i == KS - 1),
                                    )
                        o = ostp.tile([128, R, W], f32, name="o", tag="o")
                        nc.scalar.copy(o[:], ps[:])
                        nc.sync.dma_start(
                            out[b, cb * 128:(cb + 1) * 128, r0:r0 + R, :],
                            o[:],
                        )
                    r0 += R
    nc.compile()
    return nc


def _variant():
    return os.environ.get("BASS_KERNEL_VARIANT", "bf16")


def _get_nc():
    v = _variant()
    if v not in _STATE:
        _STATE[v] = _build_nc_bf16() if v == "bf16" else _build_nc()
    return _STATE[v]


def run(input, weight, trace=False):
    """Run on 8 NeuronCores; returns (output, BassKernelResults)."""
    from concourse.bass_utils import run_bass_kernel_spmd

    assert input.shape == (B, C, H, W), input.shape
    assert weight.shape == (B, CW, KK, H * W), weight.shape
    if _variant() == "bf16":
        import ml_dtypes

        dt = ml_dtypes.bfloat16
        # Pre-pad on the host: [B, C, 68*68+1] with zero borders, so the
        # kernel's two shifted SBUF copies are fully contiguous DMAs.
        inp = np.zeros((B, C, PADH * PADH + 1), dtype=dt)
        view = inp[:, :, :PADH * PADH].reshape(B, C, PADH, PADH)
        view[:, :, PAD:PAD + H, PAD:PAD + W] = np.asarray(
            input, dtype=np.float32
        ).astype(dt)
    else:
        dt = np.float32
        inp = np.ascontiguousarray(np.asarray(input, dtype=np.float32))
    wgt = np.ascontiguousarray(
        np.asarray(weight, dtype=np.float32).astype(dt).reshape(B, CW, KK, H, W)
    )
    ident = np.eye(128, dtype=dt)

    nc = _get_nc()
    core_ids = list(range(NCORES))
    in_maps = [
        {
            "input": inp[c * S:(c + 1) * S],
            "weight": wgt[c * S:(c + 1) * S],
            "ident": ident,
        }
        for c in core_ids
    ]
    res = run_bass_kernel_spmd(nc, in_maps, core_ids, trace=trace)
    outp = np.concatenate([res.results[c]["out"] for c in core_ids], axis=0)
    return outp, res


def kernel(input, weight):
    trace = bool(int(os.environ.get("BASS_KERNEL_TRACE", "0")))
    outp, _ = run(input, weight, trace=trace)
    return outp


# revision 16
# speedup vs baseline: 1.2004x; 1.2004x over previous
"""Trainium2 Bass kernel for nn_Aggregation (involution-style local aggregation).

out[b, g*64+cw, ho, wo] = sum_{i,j in 5x5} xpad[b, g*64+cw, ho+i, wo+j]
                          * weight[b, cw, i*5+j, ho*64+wo]

Data-parallel over batch: 16 samples -> 8 NeuronCores, 2 samples/core.
Per core:
  - DVE computes the 25 shifted elementwise products (batched 5 window
    shifts per tensor_tensor via an overlapping access pattern),
  - TensorE accumulates them into PSUM with identity-stationary matmuls
    (1 cycle/row),
  - ScalarE evacuates PSUM -> SBUF, DMA writes back.
"""

import os
import sys

import numpy as np

sys.path.insert(0, "/opt/trn_rl_repo")

# Problem constants (hardcoded per contract)
B, C, H, W = 16, 512, 64, 64
CW, KK, KS = 64, 25, 5
PAD = 2
NCORES = 8
S = B // NCORES          # samples per core = 2
PADH = H + 2 * PAD       # 68
NBLK = C // 128          # 4 channel blocks of 128 (each = 2 share-groups)
ROWS = 8                 # output rows per chunk
CHUNK = ROWS * W         # 512 positions = 1 PSUM bank of fp32
NCHUNK = H // ROWS       # 8 chunks per sample

_STATE = {}


def _build_nc():
    import concourse.bass as bass
    import concourse.bacc as bacc
    import concourse.tile as tile
    from concourse import mybir

    f32 = mybir.dt.float32
    f32r = mybir.dt.float32r

    nc = bacc.Bacc("TRN2", target_bir_lowering=False, debug=False, num_devices=NCORES)
    x_in = nc.declare_dram_parameter("input", [S, C, H, W], f32, isOutput=False)
    w_in = nc.declare_dram_parameter("weight", [S, CW, KK, H, W], f32, isOutput=False)
    id_in = nc.declare_dram_parameter("ident", [128, 128], f32, isOutput=False)
    out = nc.declare_dram_parameter("out", [S, C, H, W], f32, isOutput=True)

    with tile.TileContext(nc) as tc:
        with (
            tc.tile_pool(name="const", bufs=1) as constp,
            tc.tile_pool(name="xp", bufs=1) as xpp,
            tc.tile_pool(name="wt", bufs=2) as wtp,
            tc.tile_pool(name="tmp", bufs=2) as tmpp,
            tc.tile_pool(name="ost", bufs=2) as ostp,
            tc.tile_pool(name="ps", bufs=4, space="PSUM") as psp,
        ):
            ident = constp.tile([128, 128], f32)
            nc.sync.dma_start(ident[:], id_in[:])

            # Persistent padded-input tiles, one per channel block. Borders
            # are zeroed once; only the interior is rewritten per sample.
            xp = []
            for cb in range(NBLK):
                t = xpp.tile([128, PADH, PADH], f32, tag=f"xp{cb}")
                nc.vector.memset(t[:], 0.0)
                xp.append(t)

            for b in range(S):
                for cb in range(NBLK):
                    nc.sync.dma_start(
                        xp[cb][:, PAD:PAD + H, PAD:PAD + W],
                        x_in[b, cb * 128:(cb + 1) * 128],
                    )
                for k in range(NCHUNK):
                    wt = wtp.tile([128, KK, ROWS, W], f32)
                    # weight rows for this chunk on partitions 0..63, then
                    # duplicated to 64..127 (channel blocks span 2 groups
                    # sharing the same cw range).
                    nc.sync.dma_start(
                        wt[0:64], w_in[b, :, :, k * ROWS:(k + 1) * ROWS, :]
                    )
                    nc.sync.dma_start(wt[64:128], wt[0:64])
                    for cb in range(NBLK):
                        ps = psp.tile([128, ROWS, W], f32)
                        for j in range(KS):
                            t = tmpp.tile([128, KS, ROWS, W], f32)
                            # x window, batched over the 5 vertical shifts i:
                            # dims (i:5 @ PADH, r:ROWS @ PADH, c:W @ 1),
                            # base offset = (k*ROWS)*PADH + j
                            sl = xp[cb][:, k * ROWS:k * ROWS + ROWS, j:j + W]
                            xov = bass.AP(
                                sl.tensor, sl.offset,
                                [list(sl.ap[0]), [PADH, KS], [PADH, ROWS], [1, W]],
                            )
                            # weight idx = i*5+j for i in 0..5:
                            # offset j*ROWS*W, stride 5*ROWS*W over i
                            wsl = wt[:, j]
                            wov = bass.AP(
                                wsl.tensor, wsl.offset,
                                [list(wsl.ap[0]), [KS * ROWS * W, KS], [W, ROWS], [1, W]],
                            )
                            nc.vector.tensor_mul(t[:], xov, wov)
                            for i in range(KS):
                                idx = i * KS + j
                                nc.tensor.matmul(
                                    ps[:],
                                    ident[:].bitcast(f32r),
                                    t[:, i].bitcast(f32r),
                                    start=(j == 0 and i == 0),
                                    stop=(j == KS - 1 and i == KS - 1),
                                )
                        o = ostp.tile([128, ROWS, W], f32)
                        nc.scalar.copy(o[:], ps[:])
                        nc.sync.dma_start(
                            out[b, cb * 128:(cb + 1) * 128, k * ROWS:(k + 1) * ROWS, :],
                            o[:],
                        )
    nc.compile()
    return nc


def _build_nc_bf16():
    """bf16-products variant (best measured config: ~501 us/core).

    - DVE tensor_tensor runs in 2x_1P mode (2 elem/cycle/lane): every operand
      is bf16, innermost stride 1, 4B-aligned. Odd horizontal shifts j break
      4B alignment, so a second copy of the input, stored shifted by one
      element, serves the odd-j windows.
    - The host supplies the input pre-padded (zero borders, 68x68 per image)
      and pre-cast to bf16 as [S, C, 68*68+1]; the kernel streams it in
      per-chunk row-halo tiles (20 padded rows), fully contiguous transfers
      for both shifted copies.
    - Each tensor_tensor batches the 5 vertical taps of one horizontal shift
      via an overlapping access pattern (free size 5*16*64 = 5120).
    - Products are bf16; the 25-tap accumulation stays exact in fp32 PSUM via
      identity-stationary matmuls (1 cycle/row bf16; identity loads hide
      under the matmul stream). ScalarE evacuates PSUM -> SBUF -> DMA out.
    """
    import concourse.bass as bass
    import concourse.bacc as bacc
    import concourse.tile as tile
    from concourse import mybir

    f32 = mybir.dt.float32
    bf16 = mybir.dt.bfloat16
    NPAD = PADH * PADH   # 4624
    R = 16               # output rows per chunk
    NCH = H // R         # 4 chunks per sample
    HB = R // 2          # rows per PSUM half (512 fp32 = one bank)
    HALO = (R + KS - 1) * PADH  # 20 padded rows = 1360 elements

    nc = bacc.Bacc("TRN2", target_bir_lowering=False, debug=False, num_devices=NCORES)
    x_in = nc.declare_dram_parameter("input", [S, C, NPAD + 1], bf16, isOutput=False)
    w_in = nc.declare_dram_parameter("weight", [S, CW, KK, H, W], bf16, isOutput=False)
    id_in = nc.declare_dram_parameter("ident", [128, 128], bf16, isOutput=False)
    out = nc.declare_dram_parameter("out", [S, C, H, W], f32, isOutput=True)

    with tile.TileContext(nc) as tc:
        with (
            tc.tile_pool(name="const", bufs=1) as constp,
            tc.tile_pool(name="xp", bufs=2) as xpp,
            tc.tile_pool(name="wt", bufs=2) as wtp,
            tc.tile_pool(name="tmp", bufs=3) as tmpp,
            tc.tile_pool(name="ost", bufs=2) as ostp,
            tc.tile_pool(name="ps", bufs=3, space="PSUM") as psp,
        ):
            ident = constp.tile([128, 128], bf16)
            nc.sync.dma_start(ident[:], id_in[:])

            for b in range(S):
                for k in range(NCH):
                    row0 = k * R * PADH
                    xpa, xpb = [], []
                    for cb in range(NBLK):
                        csl = slice(cb * 128, (cb + 1) * 128)
                        ta = xpp.tile([128, HALO], bf16, tag=f"xpa{cb}", name=f"xpa{cb}")
                        nc.sync.dma_start(ta[:], x_in[b, csl, row0:row0 + HALO])
                        tb = xpp.tile([128, HALO], bf16, tag=f"xpb{cb}", name=f"xpb{cb}")
                        nc.sync.dma_start(tb[:], x_in[b, csl, row0 + 1:row0 + 1 + HALO])
                        xpa.append(ta)
                        xpb.append(tb)
                    wt = wtp.tile([128, KK, R, W], bf16, name="wt", tag="wt")
                    nc.sync.dma_start(wt[0:64], w_in[b, :, :, k * R:(k + 1) * R, :])
                    nc.sync.dma_start(wt[64:128], wt[0:64])
                    for cb in range(NBLK):
                        ps = psp.tile([128, R, W], f32, name="ps", tag="ps")
                        for j in range(KS):
                            t = tmpp.tile([128, KS, R, W], bf16, name="t", tag="t")
                            if j % 2 == 0:
                                xt = xpa[cb][:]
                                base = xt.offset + j
                            else:
                                xt = xpb[cb][:]
                                base = xt.offset + (j - 1)
                            xov = bass.AP(
                                xt.tensor, base,
                                [list(xt.ap[0]), [PADH, KS], [PADH, R], [1, W]],
                            )
                            wsl = wt[:, j]
                            wov = bass.AP(
                                wsl.tensor, wsl.offset,
                                [list(wsl.ap[0]), [KS * R * W, KS], [W, R], [1, W]],
                            )
                            nc.vector.tensor_mul(t[:], xov, wov)
                            for half in range(2):
                                for i in range(KS):
                                    nc.tensor.matmul(
                                        ps[:, half * HB:(half + 1) * HB],
                                        ident[:],
                                        t[:, i, half * HB:(half + 1) * HB],
                                        start=(j == 0 and i == 0),
                                        stop=(j == KS - 1 and i == KS - 1),
                                    )
                        o = ostp.tile([128, R, W], f32, name="o", tag="o")
                        nc.scalar.copy(o[:], ps[:])
                        nc.sync.dma_start(
                            out[b, cb * 128:(cb + 1) * 128, k * R:(k + 1) * R, :],
                            o[:],
                        )
    nc.compile()
    return nc


def _variant():
    return os.environ.get("BASS_KERNEL_VARIANT", "bf16")


def _get_nc():
    v = _variant()
    if v not in _STATE:
        _STATE[v] = _build_nc_bf16() if v == "bf16" else _build_nc()
    return _STATE[v]


def run(input, weight, trace=False):
    """Run on 8 NeuronCores; returns (output, BassKernelResults)."""
    from concourse.bass_utils import run_bass_kernel_spmd

    assert input.shape == (B, C, H, W), input.shape
    assert weight.shape == (B, CW, KK, H * W), weight.shape
    if _variant() == "bf16":
        import ml_dtypes

        dt = ml_dtypes.bfloat16
        # Pre-pad on the host: [B, C, 68*68+1] with zero borders, so the
        # kernel's two shifted SBUF copies are fully contiguous DMAs.
        inp = np.zeros((B, C, PADH * PADH + 1), dtype=dt)
        view = inp[:, :, :PADH * PADH].reshape(B, C, PADH, PADH)
        view[:, :, PAD:PAD + H, PAD:PAD + W] = np.asarray(
            input, dtype=np.float32
        ).astype(dt)
    else:
        dt = np.float32
        inp = np.ascontiguousarray(np.asarray(input, dtype=np.float32))
    wgt = np.ascontiguousarray(
        np.asarray(weight, dtype=np.float32).astype(dt).reshape(B, CW, KK, H, W)
    )
    ident = np.eye(128, dtype=dt)

    nc = _get_nc()
    core_ids = list(range(NCORES))
    in_maps = [
        {
            "input": inp[c * S:(c + 1) * S],
            "weight": wgt[c * S:(c + 1) * S],
            "ident": ident,
        }
        for c in core_ids
    ]
    res = run_bass_kernel_spmd(nc, in_maps, core_ids, trace=trace)
    outp = np.concatenate([res.results[c]["out"] for c in core_ids], axis=0)
    return outp, res


def kernel(input, weight):
    trace = bool(int(os.environ.get("BASS_KERNEL_TRACE", "0")))
    outp, _ = run(input, weight, trace=trace)
    return outp


# revision 17
# speedup vs baseline: 1.2081x; 1.0064x over previous
"""Trainium2 Bass kernel for nn_Aggregation (involution-style local aggregation).

out[b, g*64+cw, ho, wo] = sum_{i,j in 5x5} xpad[b, g*64+cw, ho+i, wo+j]
                          * weight[b, cw, i*5+j, ho*64+wo]

Data-parallel over batch: 16 samples -> 8 NeuronCores, 2 samples/core.
Per core:
  - DVE computes the 25 shifted elementwise products (batched 5 window
    shifts per tensor_tensor via an overlapping access pattern),
  - TensorE accumulates them into PSUM with identity-stationary matmuls
    (1 cycle/row),
  - ScalarE evacuates PSUM -> SBUF, DMA writes back.
"""

import os
import sys

import numpy as np

sys.path.insert(0, "/opt/trn_rl_repo")

# Problem constants (hardcoded per contract)
B, C, H, W = 16, 512, 64, 64
CW, KK, KS = 64, 25, 5
PAD = 2
NCORES = 8
S = B // NCORES          # samples per core = 2
PADH = H + 2 * PAD       # 68
NBLK = C // 128          # 4 channel blocks of 128 (each = 2 share-groups)
ROWS = 8                 # output rows per chunk
CHUNK = ROWS * W         # 512 positions = 1 PSUM bank of fp32
NCHUNK = H // ROWS       # 8 chunks per sample

_STATE = {}


def _build_nc():
    import concourse.bass as bass
    import concourse.bacc as bacc
    import concourse.tile as tile
    from concourse import mybir

    f32 = mybir.dt.float32
    f32r = mybir.dt.float32r

    nc = bacc.Bacc("TRN2", target_bir_lowering=False, debug=False, num_devices=NCORES)
    x_in = nc.declare_dram_parameter("input", [S, C, H, W], f32, isOutput=False)
    w_in = nc.declare_dram_parameter("weight", [S, CW, KK, H, W], f32, isOutput=False)
    id_in = nc.declare_dram_parameter("ident", [128, 128], f32, isOutput=False)
    out = nc.declare_dram_parameter("out", [S, C, H, W], f32, isOutput=True)

    with tile.TileContext(nc) as tc:
        with (
            tc.tile_pool(name="const", bufs=1) as constp,
            tc.tile_pool(name="xp", bufs=1) as xpp,
            tc.tile_pool(name="wt", bufs=2) as wtp,
            tc.tile_pool(name="tmp", bufs=2) as tmpp,
            tc.tile_pool(name="ost", bufs=2) as ostp,
            tc.tile_pool(name="ps", bufs=4, space="PSUM") as psp,
        ):
            ident = constp.tile([128, 128], f32)
            nc.sync.dma_start(ident[:], id_in[:])

            # Persistent padded-input tiles, one per channel block. Borders
            # are zeroed once; only the interior is rewritten per sample.
            xp = []
            for cb in range(NBLK):
                t = xpp.tile([128, PADH, PADH], f32, tag=f"xp{cb}")
                nc.vector.memset(t[:], 0.0)
                xp.append(t)

            for b in range(S):
                for cb in range(NBLK):
                    nc.sync.dma_start(
                        xp[cb][:, PAD:PAD + H, PAD:PAD + W],
                        x_in[b, cb * 128:(cb + 1) * 128],
                    )
                for k in range(NCHUNK):
                    wt = wtp.tile([128, KK, ROWS, W], f32)
                    # weight rows for this chunk on partitions 0..63, then
                    # duplicated to 64..127 (channel blocks span 2 groups
                    # sharing the same cw range).
                    nc.sync.dma_start(
                        wt[0:64], w_in[b, :, :, k * ROWS:(k + 1) * ROWS, :]
                    )
                    nc.sync.dma_start(wt[64:128], wt[0:64])
                    for cb in range(NBLK):
                        ps = psp.tile([128, ROWS, W], f32)
                        for j in range(KS):
                            t = tmpp.tile([128, KS, ROWS, W], f32)
                            # x window, batched over the 5 vertical shifts i:
                            # dims (i:5 @ PADH, r:ROWS @ PADH, c:W @ 1),
                            # base offset = (k*ROWS)*PADH + j
                            sl = xp[cb][:, k * ROWS:k * ROWS + ROWS, j:j + W]
                            xov = bass.AP(
                                sl.tensor, sl.offset,
                                [list(sl.ap[0]), [PADH, KS], [PADH, ROWS], [1, W]],
                            )
                            # weight idx = i*5+j for i in 0..5:
                            # offset j*ROWS*W, stride 5*ROWS*W over i
                            wsl = wt[:, j]
                            wov = bass.AP(
                                wsl.tensor, wsl.offset,
                                [list(wsl.ap[0]), [KS * ROWS * W, KS], [W, ROWS], [1, W]],
                            )
                            nc.vector.tensor_mul(t[:], xov, wov)
                            for i in range(KS):
                                idx = i * KS + j
                                nc.tensor.matmul(
                                    ps[:],
                                    ident[:].bitcast(f32r),
                                    t[:, i].bitcast(f32r),
                                    start=(j == 0 and i == 0),
                                    stop=(j == KS - 1 and i == KS - 1),
                                )
                        o = ostp.tile([128, ROWS, W], f32)
                        nc.scalar.copy(o[:], ps[:])
                        nc.sync.dma_start(
                            out[b, cb * 128:(cb + 1) * 128, k * ROWS:(k + 1) * ROWS, :],
                            o[:],
                        )
    nc.compile()
    return nc


def _build_nc_bf16():
    """bf16-products variant (best measured config: ~501 us/core).

    - DVE tensor_tensor runs in 2x_1P mode (2 elem/cycle/lane): every operand
      is bf16, innermost stride 1, 4B-aligned. Odd horizontal shifts j break
      4B alignment, so a second copy of the input, stored shifted by one
      element, serves the odd-j windows.
    - The host supplies the input pre-padded (zero borders, 68x68 per image)
      and pre-cast to bf16 as [S, C, 68*68+1]; the kernel streams it in
      per-chunk row-halo tiles (20 padded rows), fully contiguous transfers
      for both shifted copies.
    - Each tensor_tensor batches the 5 vertical taps of one horizontal shift
      via an overlapping access pattern (free size 5*16*64 = 5120).
    - Products are bf16; the 25-tap accumulation stays exact in fp32 PSUM via
      identity-stationary matmuls (1 cycle/row bf16; identity loads hide
      under the matmul stream). ScalarE evacuates PSUM -> SBUF -> DMA out.
    """
    import concourse.bass as bass
    import concourse.bacc as bacc
    import concourse.tile as tile
    from concourse import mybir

    f32 = mybir.dt.float32
    bf16 = mybir.dt.bfloat16
    NPAD = PADH * PADH   # 4624
    R = 16               # output rows per chunk
    NCH = H // R         # 4 chunks per sample
    HB = R // 2          # rows per PSUM half (512 fp32 = one bank)
    HALO = (R + KS - 1) * PADH  # 20 padded rows = 1360 elements

    nc = bacc.Bacc("TRN2", target_bir_lowering=False, debug=False, num_devices=NCORES)
    x_in = nc.declare_dram_parameter("input", [S, C, NPAD + 1], bf16, isOutput=False)
    w_in = nc.declare_dram_parameter("weight", [S, CW, KK, H, W], bf16, isOutput=False)
    id_in = nc.declare_dram_parameter("ident", [128, 128], bf16, isOutput=False)
    out = nc.declare_dram_parameter("out", [S, C, H, W], f32, isOutput=True)

    with tile.TileContext(nc) as tc:
        with (
            tc.tile_pool(name="const", bufs=1) as constp,
            tc.tile_pool(name="xp", bufs=2) as xpp,
            tc.tile_pool(name="wt", bufs=2) as wtp,
            tc.tile_pool(name="tmp", bufs=3) as tmpp,
            tc.tile_pool(name="ost", bufs=2) as ostp,
            tc.tile_pool(name="ps", bufs=3, space="PSUM") as psp,
        ):
            ident = constp.tile([128, 128], bf16)
            nc.sync.dma_start(ident[:], id_in[:])

            for b in range(S):
                for k in range(NCH):
                    row0 = k * R * PADH
                    xpa, xpb = [], []
                    # The very first chunk gates the whole pipeline: split its
                    # loads across both HWDGE queues (sync + scalar) so the
                    # first tensor_tensor starts ~2x sooner. Steady-state
                    # loads stay on sync (prefetched a chunk ahead).
                    first = (b == 0 and k == 0)
                    for cb in range(NBLK):
                        csl = slice(cb * 128, (cb + 1) * 128)
                        xa_eng = nc.scalar if (first and cb >= 2) else nc.sync
                        xb_eng = nc.scalar if first else nc.sync
                        ta = xpp.tile([128, HALO], bf16, tag=f"xpa{cb}", name=f"xpa{cb}")
                        xa_eng.dma_start(ta[:], x_in[b, csl, row0:row0 + HALO])
                        tb = xpp.tile([128, HALO], bf16, tag=f"xpb{cb}", name=f"xpb{cb}")
                        xb_eng.dma_start(tb[:], x_in[b, csl, row0 + 1:row0 + 1 + HALO])
                        xpa.append(ta)
                        xpb.append(tb)
                    wt = wtp.tile([128, KK, R, W], bf16, name="wt", tag="wt")
                    nc.sync.dma_start(wt[0:64], w_in[b, :, :, k * R:(k + 1) * R, :])
                    nc.sync.dma_start(wt[64:128], wt[0:64])
                    for cb in range(NBLK):
                        ps = psp.tile([128, R, W], f32, name="ps", tag="ps")
                        for j in range(KS):
                            t = tmpp.tile([128, KS, R, W], bf16, name="t", tag="t")
                            if j % 2 == 0:
                                xt = xpa[cb][:]
                                base = xt.offset + j
                            else:
                                xt = xpb[cb][:]
                                base = xt.offset + (j - 1)
                            xov = bass.AP(
                                xt.tensor, base,
                                [list(xt.ap[0]), [PADH, KS], [PADH, R], [1, W]],
                            )
                            wsl = wt[:, j]
                            wov = bass.AP(
                                wsl.tensor, wsl.offset,
                                [list(wsl.ap[0]), [KS * R * W, KS], [W, R], [1, W]],
                            )
                            nc.vector.tensor_mul(t[:], xov, wov)
                            for half in range(2):
                                for i in range(KS):
                                    nc.tensor.matmul(
                                        ps[:, half * HB:(half + 1) * HB],
                                        ident[:],
                                        t[:, i, half * HB:(half + 1) * HB],
                                        start=(j == 0 and i == 0),
                                        stop=(j == KS - 1 and i == KS - 1),
                                    )
                        o = ostp.tile([128, R, W], f32, name="o", tag="o")
                        nc.scalar.copy(o[:], ps[:])
                        nc.sync.dma_start(
                            out[b, cb * 128:(cb + 1) * 128, k * R:(k + 1) * R, :],
                            o[:],
                        )
    nc.compile()
    return nc


def _variant():
    return os.environ.get("BASS_KERNEL_VARIANT", "bf16")


def _get_nc():
    v = _variant()
    if v not in _STATE:
        _STATE[v] = _build_nc_bf16() if v == "bf16" else _build_nc()
    return _STATE[v]


def run(input, weight, trace=False):
    """Run on 8 NeuronCores; returns (output, BassKernelResults)."""
    from concourse.bass_utils import run_bass_kernel_spmd

    assert input.shape == (B, C, H, W), input.shape
    assert weight.shape == (B, CW, KK, H * W), weight.shape
    if _variant() == "bf16":
        import ml_dtypes

        dt = ml_dtypes.bfloat16
        # Pre-pad on the host: [B, C, 68*68+1] with zero borders, so the
        # kernel's two shifted SBUF copies are fully contiguous DMAs.
        inp = np.zeros((B, C, PADH * PADH + 1), dtype=dt)
        view = inp[:, :, :PADH * PADH].reshape(B, C, PADH, PADH)
        view[:, :, PAD:PAD + H, PAD:PAD + W] = np.asarray(
            input, dtype=np.float32
        ).astype(dt)
    else:
        dt = np.float32
        inp = np.ascontiguousarray(np.asarray(input, dtype=np.float32))
    wgt = np.ascontiguousarray(
        np.asarray(weight, dtype=np.float32).astype(dt).reshape(B, CW, KK, H, W)
    )
    ident = np.eye(128, dtype=dt)

    nc = _get_nc()
    core_ids = list(range(NCORES))
    in_maps = [
        {
            "input": inp[c * S:(c + 1) * S],
            "weight": wgt[c * S:(c + 1) * S],
            "ident": ident,
        }
        for c in core_ids
    ]
    res = run_bass_kernel_spmd(nc, in_maps, core_ids, trace=trace)
    outp = np.concatenate([res.results[c]["out"] for c in core_ids], axis=0)
    return outp, res


def kernel(input, weight):
    trace = bool(int(os.environ.get("BASS_KERNEL_TRACE", "0")))
    outp, _ = run(input, weight, trace=trace)
    return outp


# revision 18
# speedup vs baseline: 1.2093x; 1.0010x over previous
"""Trainium2 Bass kernel for nn_Aggregation (involution-style local aggregation).

out[b, g*64+cw, ho, wo] = sum_{i,j in 5x5} xpad[b, g*64+cw, ho+i, wo+j]
                          * weight[b, cw, i*5+j, ho*64+wo]

Data-parallel over batch: 16 samples -> 8 NeuronCores, 2 samples/core.
Per core:
  - DVE computes the 25 shifted elementwise products (batched 5 window
    shifts per tensor_tensor via an overlapping access pattern),
  - TensorE accumulates them into PSUM with identity-stationary matmuls
    (1 cycle/row),
  - ScalarE evacuates PSUM -> SBUF, DMA writes back.
"""

import os
import sys

import numpy as np

sys.path.insert(0, "/opt/trn_rl_repo")

# Problem constants (hardcoded per contract)
B, C, H, W = 16, 512, 64, 64
CW, KK, KS = 64, 25, 5
PAD = 2
NCORES = 8
S = B // NCORES          # samples per core = 2
PADH = H + 2 * PAD       # 68
NBLK = C // 128          # 4 channel blocks of 128 (each = 2 share-groups)
ROWS = 8                 # output rows per chunk
CHUNK = ROWS * W         # 512 positions = 1 PSUM bank of fp32
NCHUNK = H // ROWS       # 8 chunks per sample

_STATE = {}


def _build_nc():
    import concourse.bass as bass
    import concourse.bacc as bacc
    import concourse.tile as tile
    from concourse import mybir

    f32 = mybir.dt.float32
    f32r = mybir.dt.float32r

    nc = bacc.Bacc("TRN2", target_bir_lowering=False, debug=False, num_devices=NCORES)
    x_in = nc.declare_dram_parameter("input", [S, C, H, W], f32, isOutput=False)
    w_in = nc.declare_dram_parameter("weight", [S, CW, KK, H, W], f32, isOutput=False)
    id_in = nc.declare_dram_parameter("ident", [128, 128], f32, isOutput=False)
    out = nc.declare_dram_parameter("out", [S, C, H, W], f32, isOutput=True)

    with tile.TileContext(nc) as tc:
        with (
            tc.tile_pool(name="const", bufs=1) as constp,
            tc.tile_pool(name="xp", bufs=1) as xpp,
            tc.tile_pool(name="wt", bufs=2) as wtp,
            tc.tile_pool(name="tmp", bufs=2) as tmpp,
            tc.tile_pool(name="ost", bufs=2) as ostp,
            tc.tile_pool(name="ps", bufs=4, space="PSUM") as psp,
        ):
            ident = constp.tile([128, 128], f32)
            nc.sync.dma_start(ident[:], id_in[:])

            # Persistent padded-input tiles, one per channel block. Borders
            # are zeroed once; only the interior is rewritten per sample.
            xp = []
            for cb in range(NBLK):
                t = xpp.tile([128, PADH, PADH], f32, tag=f"xp{cb}")
                nc.vector.memset(t[:], 0.0)
                xp.append(t)

            for b in range(S):
                for cb in range(NBLK):
                    nc.sync.dma_start(
                        xp[cb][:, PAD:PAD + H, PAD:PAD + W],
                        x_in[b, cb * 128:(cb + 1) * 128],
                    )
                for k in range(NCHUNK):
                    wt = wtp.tile([128, KK, ROWS, W], f32)
                    # weight rows for this chunk on partitions 0..63, then
                    # duplicated to 64..127 (channel blocks span 2 groups
                    # sharing the same cw range).
                    nc.sync.dma_start(
                        wt[0:64], w_in[b, :, :, k * ROWS:(k + 1) * ROWS, :]
                    )
                    nc.sync.dma_start(wt[64:128], wt[0:64])
                    for cb in range(NBLK):
                        ps = psp.tile([128, ROWS, W], f32)
                        for j in range(KS):
                            t = tmpp.tile([128, KS, ROWS, W], f32)
                            # x window, batched over the 5 vertical shifts i:
                            # dims (i:5 @ PADH, r:ROWS @ PADH, c:W @ 1),
                            # base offset = (k*ROWS)*PADH + j
                            sl = xp[cb][:, k * ROWS:k * ROWS + ROWS, j:j + W]
                            xov = bass.AP(
                                sl.tensor, sl.offset,
                                [list(sl.ap[0]), [PADH, KS], [PADH, ROWS], [1, W]],
                            )
                            # weight idx = i*5+j for i in 0..5:
                            # offset j*ROWS*W, stride 5*ROWS*W over i
                            wsl = wt[:, j]
                            wov = bass.AP(
                                wsl.tensor, wsl.offset,
                                [list(wsl.ap[0]), [KS * ROWS * W, KS], [W, ROWS], [1, W]],
                            )
                            nc.vector.tensor_mul(t[:], xov, wov)
                            for i in range(KS):
                                idx = i * KS + j
                                nc.tensor.matmul(
                                    ps[:],
                                    ident[:].bitcast(f32r),
                                    t[:, i].bitcast(f32r),
                                    start=(j == 0 and i == 0),
                                    stop=(j == KS - 1 and i == KS - 1),
                                )
                        o = ostp.tile([128, ROWS, W], f32)
                        nc.scalar.copy(o[:], ps[:])
                        nc.sync.dma_start(
                            out[b, cb * 128:(cb + 1) * 128, k * ROWS:(k + 1) * ROWS, :],
                            o[:],
                        )
    nc.compile()
    return nc


def _build_nc_bf16():
    """bf16-products variant (best measured config: ~501 us/core).

    - DVE tensor_tensor runs in 2x_1P mode (2 elem/cycle/lane): every operand
      is bf16, innermost stride 1, 4B-aligned. Odd horizontal shifts j break
      4B alignment, so a second copy of the input, stored shifted by one
      element, serves the odd-j windows.
    - The host supplies the input pre-padded (zero borders, 68x68 per image)
      and pre-cast to bf16 as [S, C, 68*68+1]; the kernel streams it in
      per-chunk row-halo tiles (20 padded rows), fully contiguous transfers
      for both shifted copies.
    - Each tensor_tensor batches the 5 vertical taps of one horizontal shift
      via an overlapping access pattern (free size 5*16*64 = 5120).
    - Products are bf16; the 25-tap accumulation stays exact in fp32 PSUM via
      identity-stationary matmuls (1 cycle/row bf16; identity loads hide
      under the matmul stream). ScalarE evacuates PSUM -> SBUF -> DMA out.
    """
    import concourse.bass as bass
    import concourse.bacc as bacc
    import concourse.tile as tile
    from concourse import mybir

    f32 = mybir.dt.float32
    bf16 = mybir.dt.bfloat16
    NPAD = PADH * PADH   # 4624
    R = 16               # output rows per chunk
    NCH = H // R         # 4 chunks per sample
    HB = R // 2          # rows per PSUM half (512 fp32 = one bank)
    HALO = (R + KS - 1) * PADH  # 20 padded rows = 1360 elements

    nc = bacc.Bacc("TRN2", target_bir_lowering=False, debug=False, num_devices=NCORES)
    x_in = nc.declare_dram_parameter("input", [S, C, NPAD + 1], bf16, isOutput=False)
    w_in = nc.declare_dram_parameter("weight", [S, CW, KK, H, W], bf16, isOutput=False)
    id_in = nc.declare_dram_parameter("ident", [128, 128], bf16, isOutput=False)
    out = nc.declare_dram_parameter("out", [S, C, H, W], f32, isOutput=True)

    with tile.TileContext(nc) as tc:
        with (
            tc.tile_pool(name="const", bufs=1) as constp,
            tc.tile_pool(name="xp", bufs=2) as xpp,
            tc.tile_pool(name="wt", bufs=2) as wtp,
            tc.tile_pool(name="tmp", bufs=4) as tmpp,
            tc.tile_pool(name="ost", bufs=3) as ostp,
            tc.tile_pool(name="ps", bufs=4, space="PSUM") as psp,
        ):
            ident = constp.tile([128, 128], bf16)
            nc.scalar.dma_start(ident[:], id_in[:])

            for b in range(S):
                for k in range(NCH):
                    row0 = k * R * PADH
                    xpa, xpb = [], []
                    # The very first chunk gates the whole pipeline: split its
                    # loads across both HWDGE queues (sync + scalar) so the
                    # first tensor_tensor starts ~2x sooner. Steady-state
                    # loads stay on sync (prefetched a chunk ahead).
                    first = (b == 0 and k == 0)
                    for cb in range(NBLK):
                        csl = slice(cb * 128, (cb + 1) * 128)
                        xa_eng = nc.scalar if (first and cb >= 2) else nc.sync
                        xb_eng = nc.scalar if first else nc.sync
                        ta = xpp.tile([128, HALO], bf16, tag=f"xpa{cb}", name=f"xpa{cb}")
                        xa_eng.dma_start(ta[:], x_in[b, csl, row0:row0 + HALO])
                        tb = xpp.tile([128, HALO], bf16, tag=f"xpb{cb}", name=f"xpb{cb}")
                        xb_eng.dma_start(tb[:], x_in[b, csl, row0 + 1:row0 + 1 + HALO])
                        xpa.append(ta)
                        xpb.append(tb)
                    wt = wtp.tile([128, KK, R, W], bf16, name="wt", tag="wt")
                    nc.sync.dma_start(wt[0:64], w_in[b, :, :, k * R:(k + 1) * R, :])
                    nc.sync.dma_start(wt[64:128], wt[0:64])
                    for cb in range(NBLK):
                        ps = psp.tile([128, R, W], f32, name="ps", tag="ps")
                        for j in range(KS):
                            t = tmpp.tile([128, KS, R, W], bf16, name="t", tag="t")
                            if j % 2 == 0:
                                xt = xpa[cb][:]
                                base = xt.offset + j
                            else:
                                xt = xpb[cb][:]
                                base = xt.offset + (j - 1)
                            xov = bass.AP(
                                xt.tensor, base,
                                [list(xt.ap[0]), [PADH, KS], [PADH, R], [1, W]],
                            )
                            wsl = wt[:, j]
                            wov = bass.AP(
                                wsl.tensor, wsl.offset,
                                [list(wsl.ap[0]), [KS * R * W, KS], [W, R], [1, W]],
                            )
                            nc.vector.tensor_mul(t[:], xov, wov)
                            for half in range(2):
                                for i in range(KS):
                                    nc.tensor.matmul(
                                        ps[:, half * HB:(half + 1) * HB],
                                        ident[:],
                                        t[:, i, half * HB:(half + 1) * HB],
                                        start=(j == 0 and i == 0),
                                        stop=(j == KS - 1 and i == KS - 1),
                                    )
                        o = ostp.tile([128, R, W], f32, name="o", tag="o")
                        nc.scalar.copy(o[:], ps[:])
                        nc.sync.dma_start(
                            out[b, cb * 128:(cb + 1) * 128, k * R:(k + 1) * R, :],
                            o[:],
                        )
    nc.compile()
    return nc


def _variant():
    return os.environ.get("BASS_KERNEL_VARIANT", "bf16")


def _get_nc():
    v = _variant()
    if v not in _STATE:
        _STATE[v] = _build_nc_bf16() if v == "bf16" else _build_nc()
    return _STATE[v]


def run(input, weight, trace=False):
    """Run on 8 NeuronCores; returns (output, BassKernelResults)."""
    from concourse.bass_utils import run_bass_kernel_spmd

    assert input.shape == (B, C, H, W), input.shape
    assert weight.shape == (B, CW, KK, H * W), weight.shape
    if _variant() == "bf16":
        import ml_dtypes

        dt = ml_dtypes.bfloat16
        # Pre-pad on the host: [B, C, 68*68+1] with zero borders, so the
        # kernel's two shifted SBUF copies are fully contiguous DMAs.
        inp = np.zeros((B, C, PADH * PADH + 1), dtype=dt)
        view = inp[:, :, :PADH * PADH].reshape(B, C, PADH, PADH)
        view[:, :, PAD:PAD + H, PAD:PAD + W] = np.asarray(
            input, dtype=np.float32
        ).astype(dt)
    else:
        dt = np.float32
        inp = np.ascontiguousarray(np.asarray(input, dtype=np.float32))
    wgt = np.ascontiguousarray(
        np.asarray(weight, dtype=np.float32).astype(dt).reshape(B, CW, KK, H, W)
    )
    ident = np.eye(128, dtype=dt)

    nc = _get_nc()
    core_ids = list(range(NCORES))
    in_maps = [
        {
            "input": inp[c * S:(c + 1) * S],
            "weight": wgt[c * S:(c + 1) * S],
            "ident": ident,
        }
        for c in core_ids
    ]
    res = run_bass_kernel_spmd(nc, in_maps, core_ids, trace=trace)
    outp = np.concatenate([res.results[c]["out"] for c in core_ids], axis=0)
    return outp, res


def kernel(input, weight):
    trace = bool(int(os.environ.get("BASS_KERNEL_TRACE", "0")))
    outp, _ = run(input, weight, trace=trace)
    return outp


# revision 19
# speedup vs baseline: 1.2514x; 1.0348x over previous
"""Trainium2 Bass kernel for nn_Aggregation (involution-style local aggregation).

out[b, g*64+cw, ho, wo] = sum_{i,j in 5x5} xpad[b, g*64+cw, ho+i, wo+j]
                          * weight[b, cw, i*5+j, ho*64+wo]

Data-parallel over batch: 16 samples -> 8 NeuronCores, 2 samples/core.
Per core:
  - DVE computes the 25 shifted elementwise products (batched 5 window
    shifts per tensor_tensor via an overlapping access pattern),
  - TensorE accumulates them into PSUM with identity-stationary matmuls
    (1 cycle/row),
  - ScalarE evacuates PSUM -> SBUF, DMA writes back.
"""

import os
import sys

import numpy as np

sys.path.insert(0, "/opt/trn_rl_repo")

# Problem constants (hardcoded per contract)
B, C, H, W = 16, 512, 64, 64
CW, KK, KS = 64, 25, 5
PAD = 2
NCORES = 8
S = B // NCORES          # samples per core = 2
PADH = H + 2 * PAD       # 68
NBLK = C // 128          # 4 channel blocks of 128 (each = 2 share-groups)
ROWS = 8                 # output rows per chunk
CHUNK = ROWS * W         # 512 positions = 1 PSUM bank of fp32
NCHUNK = H // ROWS       # 8 chunks per sample

_STATE = {}


def _build_nc():
    import concourse.bass as bass
    import concourse.bacc as bacc
    import concourse.tile as tile
    from concourse import mybir

    f32 = mybir.dt.float32
    f32r = mybir.dt.float32r

    nc = bacc.Bacc("TRN2", target_bir_lowering=False, debug=False, num_devices=NCORES)
    x_in = nc.declare_dram_parameter("input", [S, C, H, W], f32, isOutput=False)
    w_in = nc.declare_dram_parameter("weight", [S, CW, KK, H, W], f32, isOutput=False)
    id_in = nc.declare_dram_parameter("ident", [128, 128], f32, isOutput=False)
    out = nc.declare_dram_parameter("out", [S, C, H, W], f32, isOutput=True)

    with tile.TileContext(nc) as tc:
        with (
            tc.tile_pool(name="const", bufs=1) as constp,
            tc.tile_pool(name="xp", bufs=1) as xpp,
            tc.tile_pool(name="wt", bufs=2) as wtp,
            tc.tile_pool(name="tmp", bufs=2) as tmpp,
            tc.tile_pool(name="ost", bufs=2) as ostp,
            tc.tile_pool(name="ps", bufs=4, space="PSUM") as psp,
        ):
            ident = constp.tile([128, 128], f32)
            nc.sync.dma_start(ident[:], id_in[:])

            # Persistent padded-input tiles, one per channel block. Borders
            # are zeroed once; only the interior is rewritten per sample.
            xp = []
            for cb in range(NBLK):
                t = xpp.tile([128, PADH, PADH], f32, tag=f"xp{cb}")
                nc.vector.memset(t[:], 0.0)
                xp.append(t)

            for b in range(S):
                for cb in range(NBLK):
                    nc.sync.dma_start(
                        xp[cb][:, PAD:PAD + H, PAD:PAD + W],
                        x_in[b, cb * 128:(cb + 1) * 128],
                    )
                for k in range(NCHUNK):
                    wt = wtp.tile([128, KK, ROWS, W], f32)
                    # weight rows for this chunk on partitions 0..63, then
                    # duplicated to 64..127 (channel blocks span 2 groups
                    # sharing the same cw range).
                    nc.sync.dma_start(
                        wt[0:64], w_in[b, :, :, k * ROWS:(k + 1) * ROWS, :]
                    )
                    nc.sync.dma_start(wt[64:128], wt[0:64])
                    for cb in range(NBLK):
                        ps = psp.tile([128, ROWS, W], f32)
                        for j in range(KS):
                            t = tmpp.tile([128, KS, ROWS, W], f32)
                            # x window, batched over the 5 vertical shifts i:
                            # dims (i:5 @ PADH, r:ROWS @ PADH, c:W @ 1),
                            # base offset = (k*ROWS)*PADH + j
                            sl = xp[cb][:, k * ROWS:k * ROWS + ROWS, j:j + W]
                            xov = bass.AP(
                                sl.tensor, sl.offset,
                                [list(sl.ap[0]), [PADH, KS], [PADH, ROWS], [1, W]],
                            )
                            # weight idx = i*5+j for i in 0..5:
                            # offset j*ROWS*W, stride 5*ROWS*W over i
                            wsl = wt[:, j]
                            wov = bass.AP(
                                wsl.tensor, wsl.offset,
                                [list(wsl.ap[0]), [KS * ROWS * W, KS], [W, ROWS], [1, W]],
                            )
                            nc.vector.tensor_mul(t[:], xov, wov)
                            for i in range(KS):
                                idx = i * KS + j
                                nc.tensor.matmul(
                                    ps[:],
                                    ident[:].bitcast(f32r),
                                    t[:, i].bitcast(f32r),
                                    start=(j == 0 and i == 0),
                                    stop=(j == KS - 1 and i == KS - 1),
                                )
                        o = ostp.tile([128, ROWS, W], f32)
                        nc.scalar.copy(o[:], ps[:])
                        nc.sync.dma_start(
                            out[b, cb * 128:(cb + 1) * 128, k * ROWS:(k + 1) * ROWS, :],
                            o[:],
                        )
    nc.compile()
    return nc


def _build_nc_bf16():
    """bf16-products variant (best measured config: ~501 us/core).

    - DVE tensor_tensor runs in 2x_1P mode (2 elem/cycle/lane): every operand
      is bf16, innermost stride 1, 4B-aligned. Odd horizontal shifts j break
      4B alignment, so a second copy of the input, stored shifted by one
      element, serves the odd-j windows.
    - The host supplies the input pre-padded (zero borders, 68x68 per image)
      and pre-cast to bf16 as [S, C, 68*68+1]; the kernel streams it in
      per-chunk row-halo tiles (20 padded rows), fully contiguous transfers
      for both shifted copies.
    - Each tensor_tensor batches the 5 vertical taps of one horizontal shift
      via an overlapping access pattern (free size 5*16*64 = 5120).
    - Products are bf16; the 25-tap accumulation stays exact in fp32 PSUM via
      identity-stationary matmuls (1 cycle/row bf16; identity loads hide
      under the matmul stream). ScalarE evacuates PSUM -> SBUF -> DMA out.
    """
    import concourse.bass as bass
    import concourse.bacc as bacc
    import concourse.tile as tile
    from concourse import mybir

    f32 = mybir.dt.float32
    bf16 = mybir.dt.bfloat16
    NPAD = PADH * PADH   # 4624
    R = 16               # output rows per chunk
    NCH = H // R         # 4 chunks per sample
    HB = R // 2          # rows per PSUM half (512 fp32 = one bank)
    HALO = (R + KS - 1) * PADH  # 20 padded rows = 1360 elements

    nc = bacc.Bacc("TRN2", target_bir_lowering=False, debug=False, num_devices=NCORES)
    x_in = nc.declare_dram_parameter("input", [S, C, NPAD + 1], bf16, isOutput=False)
    w_in = nc.declare_dram_parameter("weight", [S, CW, KK, H, W], bf16, isOutput=False)
    id_in = nc.declare_dram_parameter("ident", [128, 128], bf16, isOutput=False)
    out = nc.declare_dram_parameter("out", [S, C, H, W], f32, isOutput=True)

    with tile.TileContext(nc) as tc:
        with (
            tc.tile_pool(name="const", bufs=1) as constp,
            tc.tile_pool(name="xp", bufs=2) as xpp,
            tc.tile_pool(name="wt", bufs=2) as wtp,
            tc.tile_pool(name="tmp", bufs=4) as tmpp,
            tc.tile_pool(name="ost", bufs=3) as ostp,
            tc.tile_pool(name="ps", bufs=4, space="PSUM") as psp,
        ):
            ident = constp.tile([128, 128], bf16)
            nc.scalar.dma_start(ident[:], id_in[:])

            for b in range(S):
                for k in range(NCH):
                    row0 = k * R * PADH
                    # Two parallel HBM reads of the same weight rows replace
                    # the former SBUF->SBUF partition-duplication DMA, which
                    # serialized behind the x transfers on its FIFO queue and
                    # gated the first products of every chunk. x loads split
                    # across the two HWDGE queues likewise; the first chunk
                    # interleaves so cb=0's operands land first.
                    wsrc = w_in[b, :, :, k * R:(k + 1) * R, :]
                    wt = wtp.tile([128, KK, R, W], bf16, name="wt", tag="wt")
                    xpa, xpb = [], []
                    for cb in range(NBLK):
                        csl = slice(cb * 128, (cb + 1) * 128)
                        ta = xpp.tile([128, HALO], bf16, tag=f"xpa{cb}", name=f"xpa{cb}")
                        nc.sync.dma_start(ta[:], x_in[b, csl, row0:row0 + HALO])
                        tb = xpp.tile([128, HALO], bf16, tag=f"xpb{cb}", name=f"xpb{cb}")
                        nc.scalar.dma_start(tb[:], x_in[b, csl, row0 + 1:row0 + 1 + HALO])
                        xpa.append(ta)
                        xpb.append(tb)
                        if cb == 0:
                            nc.sync.dma_start(wt[0:64], wsrc)
                            nc.scalar.dma_start(wt[64:128], wsrc)
                    for cb in range(NBLK):
                        ps = psp.tile([128, R, W], f32, name="ps", tag="ps")
                        for j in range(KS):
                            t = tmpp.tile([128, KS, R, W], bf16, name="t", tag="t")
                            if j % 2 == 0:
                                xt = xpa[cb][:]
                                base = xt.offset + j
                            else:
                                xt = xpb[cb][:]
                                base = xt.offset + (j - 1)
                            xov = bass.AP(
                                xt.tensor, base,
                                [list(xt.ap[0]), [PADH, KS], [PADH, R], [1, W]],
                            )
                            wsl = wt[:, j]
                            wov = bass.AP(
                                wsl.tensor, wsl.offset,
                                [list(wsl.ap[0]), [KS * R * W, KS], [W, R], [1, W]],
                            )
                            nc.vector.tensor_mul(t[:], xov, wov)
                            for half in range(2):
                                for i in range(KS):
                                    nc.tensor.matmul(
                                        ps[:, half * HB:(half + 1) * HB],
                                        ident[:],
                                        t[:, i, half * HB:(half + 1) * HB],
                                        start=(j == 0 and i == 0),
                                        stop=(j == KS - 1 and i == KS - 1),
                                    )
                        o = ostp.tile([128, R, W], f32, name="o", tag="o")
                        nc.scalar.copy(o[:], ps[:])
                        nc.sync.dma_start(
                            out[b, cb * 128:(cb + 1) * 128, k * R:(k + 1) * R, :],
                            o[:],
                        )
    nc.compile()
    return nc


def _variant():
    return os.environ.get("BASS_KERNEL_VARIANT", "bf16")


def _get_nc():
    v = _variant()
    if v not in _STATE:
        _STATE[v] = _build_nc_bf16() if v == "bf16" else _build_nc()
    return _STATE[v]


def run(input, weight, trace=False):
    """Run on 8 NeuronCores; returns (output, BassKernelResults)."""
    from concourse.bass_utils import run_bass_kernel_spmd

    assert input.shape == (B, C, H, W), input.shape
    assert weight.shape == (B, CW, KK, H * W), weight.shape
    if _variant() == "bf16":
        import ml_dtypes

        dt = ml_dtypes.bfloat16
        # Pre-pad on the host: [B, C, 68*68+1] with zero borders, so the
        # kernel's two shifted SBUF copies are fully contiguous DMAs.
        inp = np.zeros((B, C, PADH * PADH + 1), dtype=dt)
        view = inp[:, :, :PADH * PADH].reshape(B, C, PADH, PADH)
        view[:, :, PAD:PAD + H, PAD:PAD + W] = np.asarray(
            input, dtype=np.float32
        ).astype(dt)
    else:
        dt = np.float32
        inp = np.ascontiguousarray(np.asarray(input, dtype=np.float32))
    wgt = np.ascontiguousarray(
        np.asarray(weight, dtype=np.float32).astype(dt).reshape(B, CW, KK, H, W)
    )
    ident = np.eye(128, dtype=dt)

    nc = _get_nc()
    core_ids = list(range(NCORES))
    in_maps = [
        {
            "input": inp[c * S:(c + 1) * S],
            "weight": wgt[c * S:(c + 1) * S],
            "ident": ident,
        }
        for c in core_ids
    ]
    res = run_bass_kernel_spmd(nc, in_maps, core_ids, trace=trace)
    outp = np.concatenate([res.results[c]["out"] for c in core_ids], axis=0)
    return outp, res


def kernel(input, weight):
    trace = bool(int(os.environ.get("BASS_KERNEL_TRACE", "0")))
    outp, _ = run(input, weight, trace=trace)
    return outp


# revision 20
# speedup vs baseline: 1.2543x; 1.0023x over previous
"""Trainium2 Bass kernel for nn_Aggregation (involution-style local aggregation).

out[b, g*64+cw, ho, wo] = sum_{i,j in 5x5} xpad[b, g*64+cw, ho+i, wo+j]
                          * weight[b, cw, i*5+j, ho*64+wo]

Data-parallel over batch: 16 samples -> 8 NeuronCores, 2 samples/core.
Per core:
  - DVE computes the 25 shifted elementwise products (batched 5 window
    shifts per tensor_tensor via an overlapping access pattern),
  - TensorE accumulates them into PSUM with identity-stationary matmuls
    (1 cycle/row),
  - ScalarE evacuates PSUM -> SBUF, DMA writes back.
"""

import os
import sys

import numpy as np

sys.path.insert(0, "/opt/trn_rl_repo")

# Problem constants (hardcoded per contract)
B, C, H, W = 16, 512, 64, 64
CW, KK, KS = 64, 25, 5
PAD = 2
NCORES = 8
S = B // NCORES          # samples per core = 2
PADH = H + 2 * PAD       # 68
NBLK = C // 128          # 4 channel blocks of 128 (each = 2 share-groups)
ROWS = 8                 # output rows per chunk
CHUNK = ROWS * W         # 512 positions = 1 PSUM bank of fp32
NCHUNK = H // ROWS       # 8 chunks per sample

_STATE = {}


def _build_nc():
    import concourse.bass as bass
    import concourse.bacc as bacc
    import concourse.tile as tile
    from concourse import mybir

    f32 = mybir.dt.float32
    f32r = mybir.dt.float32r

    nc = bacc.Bacc("TRN2", target_bir_lowering=False, debug=False, num_devices=NCORES)
    x_in = nc.declare_dram_parameter("input", [S, C, H, W], f32, isOutput=False)
    w_in = nc.declare_dram_parameter("weight", [S, CW, KK, H, W], f32, isOutput=False)
    id_in = nc.declare_dram_parameter("ident", [128, 128], f32, isOutput=False)
    out = nc.declare_dram_parameter("out", [S, C, H, W], f32, isOutput=True)

    with tile.TileContext(nc) as tc:
        with (
            tc.tile_pool(name="const", bufs=1) as constp,
            tc.tile_pool(name="xp", bufs=1) as xpp,
            tc.tile_pool(name="wt", bufs=2) as wtp,
            tc.tile_pool(name="tmp", bufs=2) as tmpp,
            tc.tile_pool(name="ost", bufs=2) as ostp,
            tc.tile_pool(name="ps", bufs=4, space="PSUM") as psp,
        ):
            ident = constp.tile([128, 128], f32)
            nc.sync.dma_start(ident[:], id_in[:])

            # Persistent padded-input tiles, one per channel block. Borders
            # are zeroed once; only the interior is rewritten per sample.
            xp = []
            for cb in range(NBLK):
                t = xpp.tile([128, PADH, PADH], f32, tag=f"xp{cb}")
                nc.vector.memset(t[:], 0.0)
                xp.append(t)

            for b in range(S):
                for cb in range(NBLK):
                    nc.sync.dma_start(
                        xp[cb][:, PAD:PAD + H, PAD:PAD + W],
                        x_in[b, cb * 128:(cb + 1) * 128],
                    )
                for k in range(NCHUNK):
                    wt = wtp.tile([128, KK, ROWS, W], f32)
                    # weight rows for this chunk on partitions 0..63, then
                    # duplicated to 64..127 (channel blocks span 2 groups
                    # sharing the same cw range).
                    nc.sync.dma_start(
                        wt[0:64], w_in[b, :, :, k * ROWS:(k + 1) * ROWS, :]
                    )
                    nc.sync.dma_start(wt[64:128], wt[0:64])
                    for cb in range(NBLK):
                        ps = psp.tile([128, ROWS, W], f32)
                        for j in range(KS):
                            t = tmpp.tile([128, KS, ROWS, W], f32)
                            # x window, batched over the 5 vertical shifts i:
                            # dims (i:5 @ PADH, r:ROWS @ PADH, c:W @ 1),
                            # base offset = (k*ROWS)*PADH + j
                            sl = xp[cb][:, k * ROWS:k * ROWS + ROWS, j:j + W]
                            xov = bass.AP(
                                sl.tensor, sl.offset,
                                [list(sl.ap[0]), [PADH, KS], [PADH, ROWS], [1, W]],
                            )
                            # weight idx = i*5+j for i in 0..5:
                            # offset j*ROWS*W, stride 5*ROWS*W over i
                            wsl = wt[:, j]
                            wov = bass.AP(
                                wsl.tensor, wsl.offset,
                                [list(wsl.ap[0]), [KS * ROWS * W, KS], [W, ROWS], [1, W]],
                            )
                            nc.vector.tensor_mul(t[:], xov, wov)
                            for i in range(KS):
                                idx = i * KS + j
                                nc.tensor.matmul(
                                    ps[:],
                                    ident[:].bitcast(f32r),
                                    t[:, i].bitcast(f32r),
                                    start=(j == 0 and i == 0),
                                    stop=(j == KS - 1 and i == KS - 1),
                                )
                        o = ostp.tile([128, ROWS, W], f32)
                        nc.scalar.copy(o[:], ps[:])
                        nc.sync.dma_start(
                            out[b, cb * 128:(cb + 1) * 128, k * ROWS:(k + 1) * ROWS, :],
                            o[:],
                        )
    nc.compile()
    return nc


def _build_nc_bf16():
    """bf16-products variant (best measured config: ~501 us/core).

    - DVE tensor_tensor runs in 2x_1P mode (2 elem/cycle/lane): every operand
      is bf16, innermost stride 1, 4B-aligned. Odd horizontal shifts j break
      4B alignment, so a second copy of the input, stored shifted by one
      element, serves the odd-j windows.
    - The host supplies the input pre-padded (zero borders, 68x68 per image)
      and pre-cast to bf16 as [S, C, 68*68+1]; the kernel streams it in
      per-chunk row-halo tiles (20 padded rows), fully contiguous transfers
      for both shifted copies.
    - Each tensor_tensor batches the 5 vertical taps of one horizontal shift
      via an overlapping access pattern (free size 5*16*64 = 5120).
    - Products are bf16; the 25-tap accumulation stays exact in fp32 PSUM via
      identity-stationary matmuls (1 cycle/row bf16; identity loads hide
      under the matmul stream). ScalarE evacuates PSUM -> SBUF -> DMA out.
    """
    import concourse.bass as bass
    import concourse.bacc as bacc
    import concourse.tile as tile
    from concourse import mybir

    f32 = mybir.dt.float32
    bf16 = mybir.dt.bfloat16
    NPAD = PADH * PADH   # 4624
    R = 16               # output rows per chunk
    NCH = H // R         # 4 chunks per sample
    HB = R // 2          # rows per PSUM half (512 fp32 = one bank)
    HALO = (R + KS - 1) * PADH  # 20 padded rows = 1360 elements

    nc = bacc.Bacc("TRN2", target_bir_lowering=False, debug=False, num_devices=NCORES)
    x_in = nc.declare_dram_parameter("input", [S, C, NPAD + 1], bf16, isOutput=False)
    w_in = nc.declare_dram_parameter("weight", [S, CW, KK, H, W], bf16, isOutput=False)
    id_in = nc.declare_dram_parameter("ident", [128, 128], bf16, isOutput=False)
    out = nc.declare_dram_parameter("out", [S, C, H, W], f32, isOutput=True)

    with tile.TileContext(nc) as tc:
        with (
            tc.tile_pool(name="const", bufs=1) as constp,
            tc.tile_pool(name="xp", bufs=2) as xpp,
            tc.tile_pool(name="wt", bufs=2) as wtp,
            tc.tile_pool(name="tmp", bufs=4) as tmpp,
            tc.tile_pool(name="ost", bufs=3) as ostp,
            tc.tile_pool(name="ps", bufs=4, space="PSUM") as psp,
        ):
            ident = constp.tile([128, 128], bf16)
            nc.scalar.dma_start(ident[:], id_in[:])

            for b in range(S):
                for k in range(NCH):
                    row0 = k * R * PADH
                    # Two parallel HBM reads of the same weight rows replace
                    # the former SBUF->SBUF partition-duplication DMA, which
                    # serialized behind the x transfers on its FIFO queue and
                    # gated the first products of every chunk. x loads split
                    # across the two HWDGE queues likewise; the first chunk
                    # interleaves so cb=0's operands land first.
                    wsrc = w_in[b, :, :, k * R:(k + 1) * R, :]
                    wt = wtp.tile([128, KK, R, W], bf16, name="wt", tag="wt")
                    xpa, xpb = [], []
                    for cb in range(NBLK):
                        csl = slice(cb * 128, (cb + 1) * 128)
                        ta = xpp.tile([128, HALO], bf16, tag=f"xpa{cb}", name=f"xpa{cb}")
                        nc.sync.dma_start(ta[:], x_in[b, csl, row0:row0 + HALO])
                        tb = xpp.tile([128, HALO], bf16, tag=f"xpb{cb}", name=f"xpb{cb}")
                        nc.scalar.dma_start(tb[:], x_in[b, csl, row0 + 1:row0 + 1 + HALO])
                        xpa.append(ta)
                        xpb.append(tb)
                        if cb == 0:
                            if b == 0 and k == 0:
                                # land the j=0 tap block first so the first
                                # tensor_tensor starts as soon as possible
                                nc.sync.dma_start(wt[0:64, 0:KS], wsrc[:, 0:KS])
                                nc.scalar.dma_start(wt[64:128, 0:KS], wsrc[:, 0:KS])
                                nc.sync.dma_start(wt[0:64, KS:KK], wsrc[:, KS:KK])
                                nc.scalar.dma_start(wt[64:128, KS:KK], wsrc[:, KS:KK])
                            else:
                                nc.sync.dma_start(wt[0:64], wsrc)
                                nc.scalar.dma_start(wt[64:128], wsrc)
                    for cb in range(NBLK):
                        ps = psp.tile([128, R, W], f32, name="ps", tag="ps")
                        for j in range(KS):
                            t = tmpp.tile([128, KS, R, W], bf16, name="t", tag="t")
                            if j % 2 == 0:
                                xt = xpa[cb][:]
                                base = xt.offset + j
                            else:
                                xt = xpb[cb][:]
                                base = xt.offset + (j - 1)
                            xov = bass.AP(
                                xt.tensor, base,
                                [list(xt.ap[0]), [PADH, KS], [PADH, R], [1, W]],
                            )
                            # weight is j-major on the host: taps for this j
                            # are the contiguous block wt[:, j*5:(j+1)*5]
                            wsl = wt[:, j * KS]
                            wov = bass.AP(
                                wsl.tensor, wsl.offset,
                                [list(wsl.ap[0]), [R * W, KS], [W, R], [1, W]],
                            )
                            nc.vector.tensor_mul(t[:], xov, wov)
                            for half in range(2):
                                for i in range(KS):
                                    nc.tensor.matmul(
                                        ps[:, half * HB:(half + 1) * HB],
                                        ident[:],
                                        t[:, i, half * HB:(half + 1) * HB],
                                        start=(j == 0 and i == 0),
                                        stop=(j == KS - 1 and i == KS - 1),
                                    )
                        o = ostp.tile([128, R, W], f32, name="o", tag="o")
                        nc.scalar.copy(o[:], ps[:])
                        nc.sync.dma_start(
                            out[b, cb * 128:(cb + 1) * 128, k * R:(k + 1) * R, :],
                            o[:],
                        )
    nc.compile()
    return nc


def _variant():
    return os.environ.get("BASS_KERNEL_VARIANT", "bf16")


def _get_nc():
    v = _variant()
    if v not in _STATE:
        _STATE[v] = _build_nc_bf16() if v == "bf16" else _build_nc()
    return _STATE[v]


def run(input, weight, trace=False):
    """Run on 8 NeuronCores; returns (output, BassKernelResults)."""
    from concourse.bass_utils import run_bass_kernel_spmd

    assert input.shape == (B, C, H, W), input.shape
    assert weight.shape == (B, CW, KK, H * W), weight.shape
    if _variant() == "bf16":
        import ml_dtypes

        dt = ml_dtypes.bfloat16
        # Pre-pad on the host: [B, C, 68*68+1] with zero borders, so the
        # kernel's two shifted SBUF copies are fully contiguous DMAs.
        inp = np.zeros((B, C, PADH * PADH + 1), dtype=dt)
        view = inp[:, :, :PADH * PADH].reshape(B, C, PADH, PADH)
        view[:, :, PAD:PAD + H, PAD:PAD + W] = np.asarray(
            input, dtype=np.float32
        ).astype(dt)
    else:
        dt = np.float32
        inp = np.ascontiguousarray(np.asarray(input, dtype=np.float32))
    wgt = np.asarray(weight, dtype=np.float32).astype(dt).reshape(B, CW, KK, H, W)
    if _variant() == "bf16":
        # j-major tap order: slot j*5+i holds original tap i*5+j, so each
        # horizontal shift's 5 vertical taps are contiguous in SBUF
        wgt = wgt.reshape(B, CW, KS, KS, H, W).transpose(0, 1, 3, 2, 4, 5)
        wgt = wgt.reshape(B, CW, KK, H, W)
    wgt = np.ascontiguousarray(wgt)
    ident = np.eye(128, dtype=dt)

    nc = _get_nc()
    core_ids = list(range(NCORES))
    in_maps = [
        {
            "input": inp[c * S:(c + 1) * S],
            "weight": wgt[c * S:(c + 1) * S],
            "ident": ident,
        }
        for c in core_ids
    ]
    res = run_bass_kernel_spmd(nc, in_maps, core_ids, trace=trace)
    outp = np.concatenate([res.results[c]["out"] for c in core_ids], axis=0)
    return outp, res


def kernel(input, weight):
    trace = bool(int(os.environ.get("BASS_KERNEL_TRACE", "0")))
    outp, _ = run(input, weight, trace=trace)
    return outp


# revision 21
# speedup vs baseline: 1.2733x; 1.0152x over previous
"""Trainium2 Bass kernel for nn_Aggregation (involution-style local aggregation).

out[b, g*64+cw, ho, wo] = sum_{i,j in 5x5} xpad[b, g*64+cw, ho+i, wo+j]
                          * weight[b, cw, i*5+j, ho*64+wo]

Data-parallel over batch: 16 samples -> 8 NeuronCores, 2 samples/core.
Per core:
  - DVE computes the 25 shifted elementwise products (batched 5 window
    shifts per tensor_tensor via an overlapping access pattern),
  - TensorE accumulates them into PSUM with identity-stationary matmuls
    (1 cycle/row),
  - ScalarE evacuates PSUM -> SBUF, DMA writes back.
"""

import os
import sys

import numpy as np

sys.path.insert(0, "/opt/trn_rl_repo")

# Problem constants (hardcoded per contract)
B, C, H, W = 16, 512, 64, 64
CW, KK, KS = 64, 25, 5
PAD = 2
NCORES = 8
S = B // NCORES          # samples per core = 2
PADH = H + 2 * PAD       # 68
NBLK = C // 128          # 4 channel blocks of 128 (each = 2 share-groups)
ROWS = 8                 # output rows per chunk
CHUNK = ROWS * W         # 512 positions = 1 PSUM bank of fp32
NCHUNK = H // ROWS       # 8 chunks per sample

_STATE = {}


def _build_nc():
    import concourse.bass as bass
    import concourse.bacc as bacc
    import concourse.tile as tile
    from concourse import mybir

    f32 = mybir.dt.float32
    f32r = mybir.dt.float32r

    nc = bacc.Bacc("TRN2", target_bir_lowering=False, debug=False, num_devices=NCORES)
    x_in = nc.declare_dram_parameter("input", [S, C, H, W], f32, isOutput=False)
    w_in = nc.declare_dram_parameter("weight", [S, CW, KK, H, W], f32, isOutput=False)
    id_in = nc.declare_dram_parameter("ident", [128, 128], f32, isOutput=False)
    out = nc.declare_dram_parameter("out", [S, C, H, W], f32, isOutput=True)

    with tile.TileContext(nc) as tc:
        with (
            tc.tile_pool(name="const", bufs=1) as constp,
            tc.tile_pool(name="xp", bufs=1) as xpp,
            tc.tile_pool(name="wt", bufs=2) as wtp,
            tc.tile_pool(name="tmp", bufs=2) as tmpp,
            tc.tile_pool(name="ost", bufs=2) as ostp,
            tc.tile_pool(name="ps", bufs=4, space="PSUM") as psp,
        ):
            ident = constp.tile([128, 128], f32)
            nc.sync.dma_start(ident[:], id_in[:])

            # Persistent padded-input tiles, one per channel block. Borders
            # are zeroed once; only the interior is rewritten per sample.
            xp = []
            for cb in range(NBLK):
                t = xpp.tile([128, PADH, PADH], f32, tag=f"xp{cb}")
                nc.vector.memset(t[:], 0.0)
                xp.append(t)

            for b in range(S):
                for cb in range(NBLK):
                    nc.sync.dma_start(
                        xp[cb][:, PAD:PAD + H, PAD:PAD + W],
                        x_in[b, cb * 128:(cb + 1) * 128],
                    )
                for k in range(NCHUNK):
                    wt = wtp.tile([128, KK, ROWS, W], f32)
                    # weight rows for this chunk on partitions 0..63, then
                    # duplicated to 64..127 (channel blocks span 2 groups
                    # sharing the same cw range).
                    nc.sync.dma_start(
                        wt[0:64], w_in[b, :, :, k * ROWS:(k + 1) * ROWS, :]
                    )
                    nc.sync.dma_start(wt[64:128], wt[0:64])
                    for cb in range(NBLK):
                        ps = psp.tile([128, ROWS, W], f32)
                        for j in range(KS):
                            t = tmpp.tile([128, KS, ROWS, W], f32)
                            # x window, batched over the 5 vertical shifts i:
                            # dims (i:5 @ PADH, r:ROWS @ PADH, c:W @ 1),
                            # base offset = (k*ROWS)*PADH + j
                            sl = xp[cb][:, k * ROWS:k * ROWS + ROWS, j:j + W]
                            xov = bass.AP(
                                sl.tensor, sl.offset,
                                [list(sl.ap[0]), [PADH, KS], [PADH, ROWS], [1, W]],
                            )
                            # weight idx = i*5+j for i in 0..5:
                            # offset j*ROWS*W, stride 5*ROWS*W over i
                            wsl = wt[:, j]
                            wov = bass.AP(
                                wsl.tensor, wsl.offset,
                                [list(wsl.ap[0]), [KS * ROWS * W, KS], [W, ROWS], [1, W]],
                            )
                            nc.vector.tensor_mul(t[:], xov, wov)
                            for i in range(KS):
                                idx = i * KS + j
                                nc.tensor.matmul(
                                    ps[:],
                                    ident[:].bitcast(f32r),
                                    t[:, i].bitcast(f32r),
                                    start=(j == 0 and i == 0),
                                    stop=(j == KS - 1 and i == KS - 1),
                                )
                        o = ostp.tile([128, ROWS, W], f32)
                        nc.scalar.copy(o[:], ps[:])
                        nc.sync.dma_start(
                            out[b, cb * 128:(cb + 1) * 128, k * ROWS:(k + 1) * ROWS, :],
                            o[:],
                        )
    nc.compile()
    return nc


def _build_nc_bf16():
    """bf16-products variant (best measured config: ~501 us/core).

    - DVE tensor_tensor runs in 2x_1P mode (2 elem/cycle/lane): every operand
      is bf16, innermost stride 1, 4B-aligned. Odd horizontal shifts j break
      4B alignment, so a second copy of the input, stored shifted by one
      element, serves the odd-j windows.
    - The host supplies the input pre-padded (zero borders, 68x68 per image)
      and pre-cast to bf16 as [S, C, 68*68+1]; the kernel streams it in
      per-chunk row-halo tiles (20 padded rows), fully contiguous transfers
      for both shifted copies.
    - Each tensor_tensor batches the 5 vertical taps of one horizontal shift
      via an overlapping access pattern (free size 5*16*64 = 5120).
    - Products are bf16; the 25-tap accumulation stays exact in fp32 PSUM via
      identity-stationary matmuls (1 cycle/row bf16; identity loads hide
      under the matmul stream). ScalarE evacuates PSUM -> SBUF -> DMA out.
    """
    import concourse.bass as bass
    import concourse.bacc as bacc
    import concourse.tile as tile
    from concourse import mybir

    f32 = mybir.dt.float32
    bf16 = mybir.dt.bfloat16
    NPAD = PADH * PADH   # 4624
    R = 16               # output rows per chunk
    NCH = H // R         # 4 chunks per sample
    HB = R // 2          # rows per PSUM half (512 fp32 = one bank)
    HALO = (R + KS - 1) * PADH  # 20 padded rows = 1360 elements

    nc = bacc.Bacc("TRN2", target_bir_lowering=False, debug=False, num_devices=NCORES)
    x_in = nc.declare_dram_parameter("input", [S, C, NPAD + 1], bf16, isOutput=False)
    w_in = nc.declare_dram_parameter("weight", [S, CW, KK, H, W], bf16, isOutput=False)
    id_in = nc.declare_dram_parameter("ident", [128, 128], bf16, isOutput=False)
    out = nc.declare_dram_parameter("out", [S, C, H, W], f32, isOutput=True)

    with tile.TileContext(nc) as tc:
        with (
            tc.tile_pool(name="const", bufs=1) as constp,
            tc.tile_pool(name="xp", bufs=2) as xpp,
            tc.tile_pool(name="wt", bufs=2) as wtp,
            tc.tile_pool(name="tmp", bufs=4) as tmpp,
            tc.tile_pool(name="ost", bufs=3) as ostp,
            tc.tile_pool(name="ps", bufs=4, space="PSUM") as psp,
        ):
            ident = constp.tile([128, 128], bf16)
            nc.scalar.dma_start(ident[:], id_in[:])

            for b in range(S):
                for k in range(NCH):
                    row0 = k * R * PADH
                    # Two parallel HBM reads of the same weight rows replace
                    # the former SBUF->SBUF partition-duplication DMA, which
                    # serialized behind the x transfers on its FIFO queue and
                    # gated the first products of every chunk. x loads split
                    # across the two HWDGE queues likewise. The very first
                    # chunk streams everything in exact consumption order
                    # (cb0's x, then the 5 tap blocks just-in-time, then the
                    # remaining channel blocks' x) so the DVE product stream
                    # starts ~10us in and never stalls.
                    wsrc = w_in[b, :, :, k * R:(k + 1) * R, :]
                    wt = wtp.tile([128, KK, R, W], bf16, name="wt", tag="wt")
                    xpa, xpb = [], []
                    for cb in range(NBLK):
                        csl = slice(cb * 128, (cb + 1) * 128)
                        ta = xpp.tile([128, HALO], bf16, tag=f"xpa{cb}", name=f"xpa{cb}")
                        tb = xpp.tile([128, HALO], bf16, tag=f"xpb{cb}", name=f"xpb{cb}")
                        xpa.append(ta)
                        xpb.append(tb)
                    first = (b == 0 and k == 0)
                    def load_x(cb):
                        csl = slice(cb * 128, (cb + 1) * 128)
                        nc.sync.dma_start(xpa[cb][:], x_in[b, csl, row0:row0 + HALO])
                        nc.scalar.dma_start(
                            xpb[cb][:], x_in[b, csl, row0 + 1:row0 + 1 + HALO]
                        )
                    if first:
                        load_x(0)
                        for j in range(KS):
                            jb = slice(j * KS, (j + 1) * KS)
                            nc.sync.dma_start(wt[0:64, jb], wsrc[:, jb])
                            nc.scalar.dma_start(wt[64:128, jb], wsrc[:, jb])
                        for cb in range(1, NBLK):
                            load_x(cb)
                    else:
                        for cb in range(NBLK):
                            load_x(cb)
                            if cb == 0:
                                nc.sync.dma_start(wt[0:64], wsrc)
                                nc.scalar.dma_start(wt[64:128], wsrc)
                    for cb in range(NBLK):
                        ps = psp.tile([128, R, W], f32, name="ps", tag="ps")
                        for j in range(KS):
                            t = tmpp.tile([128, KS, R, W], bf16, name="t", tag="t")
                            if j % 2 == 0:
                                xt = xpa[cb][:]
                                base = xt.offset + j
                            else:
                                xt = xpb[cb][:]
                                base = xt.offset + (j - 1)
                            xov = bass.AP(
                                xt.tensor, base,
                                [list(xt.ap[0]), [PADH, KS], [PADH, R], [1, W]],
                            )
                            # weight is j-major on the host: taps for this j
                            # are the contiguous block wt[:, j*5:(j+1)*5]
                            wsl = wt[:, j * KS]
                            wov = bass.AP(
                                wsl.tensor, wsl.offset,
                                [list(wsl.ap[0]), [R * W, KS], [W, R], [1, W]],
                            )
                            nc.vector.tensor_mul(t[:], xov, wov)
                            for half in range(2):
                                for i in range(KS):
                                    nc.tensor.matmul(
                                        ps[:, half * HB:(half + 1) * HB],
                                        ident[:],
                                        t[:, i, half * HB:(half + 1) * HB],
                                        start=(j == 0 and i == 0),
                                        stop=(j == KS - 1 and i == KS - 1),
                                    )
                        o = ostp.tile([128, R, W], f32, name="o", tag="o")
                        nc.scalar.copy(o[:], ps[:])
                        nc.sync.dma_start(
                            out[b, cb * 128:(cb + 1) * 128, k * R:(k + 1) * R, :],
                            o[:],
                        )
    nc.compile()
    return nc


def _variant():
    return os.environ.get("BASS_KERNEL_VARIANT", "bf16")


def _get_nc():
    v = _variant()
    if v not in _STATE:
        _STATE[v] = _build_nc_bf16() if v == "bf16" else _build_nc()
    return _STATE[v]


def run(input, weight, trace=False):
    """Run on 8 NeuronCores; returns (output, BassKernelResults)."""
    from concourse.bass_utils import run_bass_kernel_spmd

    assert input.shape == (B, C, H, W), input.shape
    assert weight.shape == (B, CW, KK, H * W), weight.shape
    if _variant() == "bf16":
        import ml_dtypes

        dt = ml_dtypes.bfloat16
        # Pre-pad on the host: [B, C, 68*68+1] with zero borders, so the
        # kernel's two shifted SBUF copies are fully contiguous DMAs.
        inp = np.zeros((B, C, PADH * PADH + 1), dtype=dt)
        view = inp[:, :, :PADH * PADH].reshape(B, C, PADH, PADH)
        view[:, :, PAD:PAD + H, PAD:PAD + W] = np.asarray(
            input, dtype=np.float32
        ).astype(dt)
    else:
        dt = np.float32
        inp = np.ascontiguousarray(np.asarray(input, dtype=np.float32))
    wgt = np.asarray(weight, dtype=np.float32).astype(dt).reshape(B, CW, KK, H, W)
    if _variant() == "bf16":
        # j-major tap order: slot j*5+i holds original tap i*5+j, so each
        # horizontal shift's 5 vertical taps are contiguous in SBUF
        wgt = wgt.reshape(B, CW, KS, KS, H, W).transpose(0, 1, 3, 2, 4, 5)
        wgt = wgt.reshape(B, CW, KK, H, W)
    wgt = np.ascontiguousarray(wgt)
    ident = np.eye(128, dtype=dt)

    nc = _get_nc()
    core_ids = list(range(NCORES))
    in_maps = [
        {
            "input": inp[c * S:(c + 1) * S],
            "weight": wgt[c * S:(c + 1) * S],
            "ident": ident,
        }
        for c in core_ids
    ]
    res = run_bass_kernel_spmd(nc, in_maps, core_ids, trace=trace)
    outp = np.concatenate([res.results[c]["out"] for c in core_ids], axis=0)
    return outp, res


def kernel(input, weight):
    trace = bool(int(os.environ.get("BASS_KERNEL_TRACE", "0")))
    outp, _ = run(input, weight, trace=trace)
    return outp
